# revision 32
# baseline (speedup 1.0000x reference)
"""DHASPI level-loss kernel for 8 Trainium2 NeuronCores.

Data-parallel over the fused B*C row axis: each core processes 64 rows of
x_env (SBUF partitions 0-63) and 64 rows of y_env (partitions 64-127). Per
row the kernel computes the gated LUFS loudness; the final relu-diff scalar
sum over the 512 rows is done on the host from the 8 tiny [128, 1] outputs.

Math notes:
- Frame energies (9600-sample windows, shift 2880) are built from 960-sample
  block sums: gcd(9600, 2880) = 960, frame f = blocks 3f..3f+9. Only blocks
  0..198 are covered by any frame, so the last 960 samples of every row are
  never loaded (191040 of 192000 samples).
- All dB-domain gating comparisons are done in the energy (frame-sum) domain
  via the monotone map el = -0.691 + 10*log10(z + eps), so the only
  transcendental on device is one Ln per row at the end.
- bf16 squares put ~2e-4 relative noise on block sums (~0.001 dB on LUFS);
  the gating margins on this problem are >9 dB, far from any flip.

Engine-balanced dataflow (all four queues ~equal):
- Loads split between the SP queue (f32) and the GpSimd/Pool queue (SWDGE
  f32->bf16 cast during DMA, half the SBUF write traffic).
- Squares split between ACT (activation Square) and DVE (tensor_tensor mult
  in bf16, 2x perf mode). Squares are IN-PLACE: the input tile is squared
  into itself (for f32 inputs, into the tile's low half via a bf16 bitcast
  view - the bf16 write offset 2i trails the f32 read offset 4i, so the
  stream never clobbers unread data).
- Block sums: one bf16 tensor_scalar per 960-block with accum_out into the
  f32 block-sum tile. bf16 single-source tensor_scalar streams at 4x
  (0.26 ns/elem) and the accumulator is fp32, so this replaces a whole
  fold-tree at a third of the cost and with no extra rounding.

Raw Bass (explicit semaphores); cross-engine RAW uses drain() before
then_inc, per-slot DMA semaphores, in-order queues carry the rest.
"""

import math

import numpy as np

import concourse.bass as bass
from concourse import mybir
from concourse.bass_utils import run_bass_kernel_spmd

# Problem constants (hardcoded from the spec; kernel.py must be self-contained)
B, C, T = 16, 32, 192000
N_CORES = 8
ROWS = B * C  # 512
RPC = ROWS // N_CORES  # 64 rows per core per tensor

FRAME = 9600
SHIFT = 2880
BLK = 960
NBLK_USED = 199  # blocks 0..198 feed frames; block 199 is dead
NFRM = (T - FRAME) // SHIFT + 1  # 64

EPS = 1e-8
ALPHA = 1e-4
GAMMA_A = -70.0
TA = float(10.0 ** ((GAMMA_A + 0.691) / 10.0) - EPS)  # z-domain abs threshold
TR_OFF = float(0.1 * EPS - EPS)
LN10_INV10 = float(10.0 / math.log(10.0))
INV_FRAME = float(1.0 / FRAME)
# The gating epilogue runs directly on frame *sums* (zsum = z * FRAME):
# thresholds and the final log are rescaled by FRAME so no divide is needed.
TA_Z = float(TA * FRAME)
TR_OFF_Z = float(TR_OFF * FRAME)
EPS_Z = float(EPS * FRAME)
FINAL_C = float(-0.691 - LN10_INV10 * math.log(FRAME))

F32 = mybir.dt.float32
BF16 = mybir.dt.bfloat16

# ---- chunk schedule -------------------------------------------------------
# sizes in blocks; 28 chunks totalling 199 blocks. Small chunks at the start
# (fast pipeline fill from cheap Pool cast loads) and at the end (short
# post-last-DMA critical path: tiny DVE square + block sums, then epilogue).
CHUNK_BLOCKS = [3, 3, 3, 4] + [7] * 25 + [7, 4]
# which chunks the SP queue loads as f32 (the rest are Pool bf16-cast loads)
SP_LOAD = {3, 6, 9, 12, 15, 18, 21, 24, 27}
# which chunks DVE squares (must be Pool-cast loads); the rest ACT squares
DVE_SQ = {1, 2, 4, 5, 8, 11, 14, 16, 17, 20, 23, 26}
# DVE processes chunks in this order (early minis first so DVE starts on its
# own squares while ACT's first square is still in flight)
DVE_ORDER = [1, 2, 0] + list(range(3, len(CHUNK_BLOCKS)))

# Pool load order: DVE's first chunks land before ACT's (ACT has end slack)
POOL_FIRST = [0, 1, 2]

N_F32 = 3  # f32 input tile slots (SP loads)
N_B16 = 6  # bf16 input tile slots (Pool cast loads)
MAXW = 7 * BLK  # widest chunk


def _sched():
    n = len(CHUNK_BLOCKS)
    off = [sum(CHUNK_BLOCKS[:i]) for i in range(n)]
    sp_list = [c for c in range(n) if c in SP_LOAD]
    pool_list = [c for c in range(n) if c not in SP_LOAD]
    pool_list = POOL_FIRST + [c for c in pool_list if c not in POOL_FIRST]
    act_sq = [c for c in range(n) if c not in DVE_SQ]
    return n, off, sp_list, pool_list, act_sq


def _frames_view(bs_ap):
    """[128, NFRM, 10] view of the block-sum tile: frame f = blocks 3f..3f+9."""
    base = bs_ap[:, 0:1]
    return type(base)(
        tensor=base.tensor,
        offset=base.offset,
        ap=[list(base.ap[0]), [3, NFRM], [1, FRAME // BLK]],
    )


def _build_program() -> bass.Bass:
    nc = bass.Bass("TRN2", target_bir_lowering=False, debug=False)
    AF = mybir.ActivationFunctionType
    ALU = mybir.AluOpType
    AX = mybir.AxisListType

    n, off, sp_list, pool_list, act_sq = _sched()
    act_ord = {c: i for i, c in enumerate(act_sq)}
    sp_ord = {c: i for i, c in enumerate(sp_list)}
    pool_ord = {c: i for i, c in enumerate(pool_list)}
    assert sorted(DVE_ORDER) == list(range(n))
    red_pos = {c: i for i, c in enumerate(DVE_ORDER)}  # s_red ordinal per chunk

    xy = nc.dram_tensor("xy", [128, T], F32, kind="ExternalInput").ap()
    out = nc.dram_tensor("lufs", [128, 1], F32, kind="ExternalOutput").ap()

    xt32 = [
        nc.alloc_sbuf_tensor(f"xt32_{i}", [128, MAXW], F32).ap() for i in range(N_F32)
    ]
    xt16 = [
        nc.alloc_sbuf_tensor(f"xt16_{i}", [128, MAXW], BF16).ap() for i in range(N_B16)
    ]
    bs = nc.alloc_sbuf_tensor("bs", [128, NBLK_USED], F32).ap()
    junk16 = nc.alloc_sbuf_tensor("junk16", [128, BLK], BF16).ap()
    zsum = nc.alloc_sbuf_tensor("zsum", [128, NFRM], F32).ap()
    ga = nc.alloc_sbuf_tensor("ga", [128, NFRM], F32).ap()
    ma = nc.alloc_sbuf_tensor("ma", [128, NFRM], F32).ap()
    gar = nc.alloc_sbuf_tensor("gar", [128, NFRM], F32).ap()
    junk = nc.alloc_sbuf_tensor("junk", [128, NFRM], F32).ap()
    sc = nc.alloc_sbuf_tensor("sc", [128, 12], F32).ap()
    eps_t = nc.alloc_sbuf_tensor("eps_t", [128, 1], F32).ap()

    numa = sc[:, 0:1]
    dena = sc[:, 1:2]
    rca = sc[:, 2:3]
    zavea = sc[:, 3:4]
    thr = sc[:, 4:5]
    denar = sc[:, 5:6]
    numar = sc[:, 6:7]
    rcar = sc[:, 7:8]
    zavear = sc[:, 8:9]
    lnz = sc[:, 9:10]
    lufs_t = sc[:, 10:11]

    def sq_tile(c):
        """bf16 view holding chunk c's squares (in-place in its input tile)."""
        if c in SP_LOAD:
            return xt32[sp_ord[c] % N_F32].bitcast(BF16)
        return xt16[pool_ord[c] % N_B16]

    with (
        nc.Block() as block,
        nc.semaphore("s_f32_0") as s_f32_0,
        nc.semaphore("s_f32_1") as s_f32_1,
        nc.semaphore("s_f32_2") as s_f32_2,
        nc.semaphore("s_f32_3") as s_f32_3,
        nc.semaphore("s_b16_0") as s_b16_0,
        nc.semaphore("s_b16_1") as s_b16_1,
        nc.semaphore("s_b16_2") as s_b16_2,
        nc.semaphore("s_b16_3") as s_b16_3,
        nc.semaphore("s_b16_4") as s_b16_4,
        nc.semaphore("s_b16_5") as s_b16_5,
        nc.semaphore("s_b16_6") as s_b16_6,
        nc.semaphore("s_sqA") as s_sqA,
        nc.semaphore("s_red") as s_red,
        nc.semaphore("s_zav") as s_zav,
        nc.semaphore("s_out") as s_out,
    ):
        s_f32 = [s_f32_0, s_f32_1, s_f32_2, s_f32_3][:N_F32]
        s_b16 = [s_b16_0, s_b16_1, s_b16_2, s_b16_3, s_b16_4, s_b16_5, s_b16_6][:N_B16]

        # ---- SP: f32 loads + final output DMA -----------------------------
        @block.sync
        def _(sy):
            for i, c in enumerate(sp_list):
                w = CHUNK_BLOCKS[c] * BLK
                slot = i % N_F32
                if i >= N_F32:
                    # slot free once its previous occupant's chunk was summed
                    sy.wait_ge(s_red, red_pos[sp_list[i - N_F32]] + 1)
                sy.dma_start(
                    out=xt32[slot][:, 0:w], in_=xy[:, off[c] * BLK : off[c] * BLK + w]
                ).then_inc(s_f32[slot], 16)
            sy.wait_ge(s_zav, 1)
            sy.dma_start(out=out, in_=zavear).then_inc(s_out, 16)
            sy.wait_ge(s_out, 16)

        # ---- Pool: bf16 cast loads ----------------------------------------
        @block.gpsimd
        def _(g):
            for i, c in enumerate(pool_list):
                w = CHUNK_BLOCKS[c] * BLK
                slot = i % N_B16
                if i >= N_B16:
                    g.wait_ge(s_red, red_pos[pool_list[i - N_B16]] + 1)
                g.dma_start(
                    out=xt16[slot][:, 0:w], in_=xy[:, off[c] * BLK : off[c] * BLK + w]
                ).then_inc(s_b16[slot], 16)

        # ---- ACT: in-place squares + final Ln/affine ----------------------
        @block.scalar
        def _(s):
            for i, c in enumerate(act_sq):
                w = CHUNK_BLOCKS[c] * BLK
                if c in SP_LOAD:
                    si = sp_ord[c]
                    s.wait_ge(s_f32[si % N_F32], (si // N_F32 + 1) * 16)
                    src = xt32[si % N_F32][:, 0:w]
                else:
                    pi = pool_ord[c]
                    s.wait_ge(s_b16[pi % N_B16], (pi // N_B16 + 1) * 16)
                    src = xt16[pi % N_B16][:, 0:w]
                s.activation(sq_tile(c)[:, 0:w], src, AF.Square)
                s.drain().then_inc(s_sqA, 1)


        # ---- DVE: bf16 squares, per-block accum sums, gating epilogue -----
        @block.vector
        def _(v):
            for c in DVE_ORDER:
                nb = CHUNK_BLOCKS[c]
                w = nb * BLK
                t = sq_tile(c)
                if c in DVE_SQ:
                    pi = pool_ord[c]
                    v.wait_ge(s_b16[pi % N_B16], (pi // N_B16 + 1) * 16)
                    v.tensor_tensor(t[:, 0:w], t[:, 0:w], t[:, 0:w], op=ALU.mult)
                    v.drain()
                else:
                    v.wait_ge(s_sqA, act_ord[c] + 1)
                # one 4x-rate bf16 tensor_scalar per block, fp32 accum -> bs
                for b in range(nb):
                    ts = v.tensor_scalar(
                        junk16, t[:, b * BLK : (b + 1) * BLK], 1.0, 0.0,
                        op0=ALU.mult, op1=ALU.add,
                        accum_out=bs[:, off[c] + b : off[c] + b + 1],
                    )
                # s_red only gates tile reuse (WAR): the last tensor_scalar's
                # reads are done at retire, so the inc rides on it directly
                ts.then_inc(s_red, 1)

            # ---- gating epilogue, all in the zsum (= z * FRAME) domain ----
            v.reduce_sum(zsum[:, :], _frames_view(bs), axis=AX.X)
            v.drain()
            v.scalar_tensor_tensor(
                out=ga[:, :], in0=zsum[:, :], scalar=TA_Z, in1=zsum[:, :],
                op0=ALU.is_gt, op1=ALU.mult,
            )
            v.tensor_scalar(ma[:, :], zsum[:, :], TA_Z, None, op0=ALU.is_gt)
            v.drain()
            v.reduce_sum(numa, ga[:, :], axis=AX.X)
            v.reduce_sum(dena, ma[:, :], axis=AX.X)
            v.drain()
            v.reciprocal(rca, dena)
            v.drain()
            v.tensor_tensor(zavea, numa, rca, op=ALU.mult)
            v.drain()
            v.tensor_scalar(thr, zavea, 0.1, TR_OFF_Z, op0=ALU.mult, op1=ALU.add)
            v.drain()
            v.scalar_tensor_tensor(
                out=gar[:, :], in0=zsum[:, :], scalar=thr, in1=ma[:, :],
                op0=ALU.is_gt, op1=ALU.mult,
            )
            v.drain()
            # zsum*gar = (zsum > thr) * ga, so reuse ga, no fresh multiply
            v.scalar_tensor_tensor(
                out=junk[:, :], in0=zsum[:, :], scalar=thr, in1=ga[:, :],
                op0=ALU.is_gt, op1=ALU.mult,
            )
            v.reduce_sum(denar, gar[:, :], axis=AX.X)
            v.drain()
            v.reduce_sum(numar, junk[:, :], axis=AX.X)
            v.drain()
            v.reciprocal(rcar, denar)
            v.drain()
            v.tensor_tensor(zavear, numar, rcar, op=ALU.mult)
            v.drain().then_inc(s_zav, 1)

    return nc


def make_in_maps(x_env: np.ndarray, y_env: np.ndarray) -> list[dict[str, np.ndarray]]:
    x = np.asarray(x_env, dtype=np.float32).reshape(ROWS, T)
    y = np.asarray(y_env, dtype=np.float32).reshape(ROWS, T)
    in_maps = []
    for i in range(N_CORES):
        shard = np.concatenate(
            [x[i * RPC : (i + 1) * RPC], y[i * RPC : (i + 1) * RPC]], axis=0
        )
        in_maps.append({"xy": np.ascontiguousarray(shard)})
    return in_maps


def finish(per_core_zav: list[np.ndarray]) -> np.ndarray:
    """per-core [128,1] gated mean frame-sums -> loss (host applies the log)."""
    total = 0.0
    for zv in per_core_zav:
        zv = np.asarray(zv).reshape(128).astype(np.float64)
        lf = LN10_INV10 * np.log(zv + EPS_Z) + FINAL_C
        total += np.maximum(lf[RPC:] - lf[:RPC], 0.0).sum()
    return np.array(ALPHA * total, dtype=np.float32)


def kernel(x_env: np.ndarray, y_env: np.ndarray) -> np.ndarray:
    nc = _build_program()
    in_maps = make_in_maps(x_env, y_env)
    res = run_bass_kernel_spmd(nc, in_maps, core_ids=list(range(N_CORES)))
    return finish([res.results[i]["lufs"] for i in range(N_CORES)])
